# revision 13
# baseline (speedup 1.0000x reference)
"""Trainium2 Bass kernel for LLM adapter attention (QK-RMSNorm + dual RoPE + SDPA).

Sharding: 8 cores = (batch b, head-group hg): core c -> b = c//2, heads
hg*8..hg*8+8 (hg = c%2).  Each core computes q/k/v projections for its 8
heads on its batch, attention, and a partial o_proj over its heads.  Host
sums the two partials per batch.

All matmuls bf16 (fp8 DoubleRow loses on either precision or LDWEIGHTS
bandwidth for this shape).  vs the first baseline: q/k stored bf16 (enables
FWL on the scores stationary), exp batched over two PSUM banks per
activation, rstd computed as Exp(-0.5*Ln(ss+eps)) so every activation
function lives in one HW table (no table reloads), wo resident per n-chunk
with double buffering, and emission is software-pipelined via generators:
projections of head-pair hp+1 interleave with attention of hp so the PE
never stalls on the scalar engine's exp stream; o_proj starts as soon as
the last head's attention block for an l-range completes.
"""

import os
import sys

import ml_dtypes
import numpy as np

for _p in ("/opt/trn_rl_repo", "/root/.axon_site/_ro/trn_rl_repo"):
    if _p not in sys.path and os.path.isdir(_p):
        sys.path.insert(0, _p)

def _install_ntff_hook_shim():
    """The agent image lacks ``antenv.axon_hooks``; synthesize it and wire
    the ctypes NTFF profiling hook so trace=True works under axon."""
    try:
        import antenv.axon_hooks  # noqa: F401
        return
    except ImportError:
        pass
    import types

    try:
        import antenv
    except ImportError:
        return
    mod = types.ModuleType("antenv.axon_hooks")
    mod._hook = None
    mod.set_axon_ntff_profile_hook = lambda h: setattr(mod, "_hook", h)
    mod.get_axon_ntff_profile_hook = lambda: mod._hook
    sys.modules["antenv.axon_hooks"] = mod
    antenv.axon_hooks = mod
    try:
        from trn_agent_boot.trn_boot import _ntff_profile_via_ctypes

        hook = _ntff_profile_via_ctypes("/opt/axon/libaxon_pjrt.so")
        if hook is not None:
            mod._hook = hook
    except Exception:
        pass


_install_ntff_hook_shim()

import concourse.bass as bass  # noqa: E402
import concourse.mybir as mybir  # noqa: E402
from concourse import bacc  # noqa: E402
from concourse.bass_utils import run_bass_kernel_spmd  # noqa: E402
from concourse.tile import TileContext  # noqa: E402

B, L, D = 4, 2048, 2048
NH, DH = 16, 128
EPS = 1e-6
P = 128
HPC = 8            # heads per core
NCORES = 8
KT = D // P        # 16 k-tiles for projections
LCP = 512          # l-chunk for projections
NLCP = L // LCP    # 4
LCA = 512          # l-chunk for attention
NLCA = L // LCA    # 4
MT = L // P        # 16 key tiles
LT = L // P        # 16 l-tiles
F32 = mybir.dt.float32
BF16 = mybir.dt.bfloat16
AF = mybir.ActivationFunctionType

_cache = {}


def _build_program():
    nc = bacc.Bacc(
        "TRN2",
        target_bir_lowering=False,
        debug=False,
        enable_asserts=False,
        num_devices=NCORES,
    )

    xT = nc.dram_tensor("xT", [D, L], BF16, kind="ExternalInput").ap()
    wqT = nc.dram_tensor("wqT", [D, HPC * DH], BF16, kind="ExternalInput").ap()
    wkT = nc.dram_tensor("wkT", [D, HPC * DH], BF16, kind="ExternalInput").ap()
    wvT = nc.dram_tensor("wvT", [D, HPC * DH], BF16, kind="ExternalInput").ap()
    woT = nc.dram_tensor("woT", [HPC * DH, D], BF16, kind="ExternalInput").ap()
    # A/B rope tensors, [4, DH, L]: 0=Aq 1=Bq 2=Ak 3=Bk
    ab = nc.dram_tensor("ab", [4, DH, L], BF16, kind="ExternalInput").ap()
    out = nc.dram_tensor("out", [L, D], F32, kind="ExternalOutput").ap()

    xv = xT.rearrange("(ko p) l -> p ko l", p=P)        # [128, 16, 2048]
    wqv = wqT.rearrange("(ko p) d -> p ko d", p=P)
    wkv = wkT.rearrange("(ko p) d -> p ko d", p=P)
    wvv = wvT.rearrange("(ko p) d -> p ko d", p=P)
    wov = woT.rearrange("(ho p) n -> p ho n", p=P)       # [128, 8, 2048]
    abv = ab.rearrange("t p l -> p t l")                 # [128, 4, 2048]
    outv = out.rearrange("(lt p) n -> p lt n", p=P)      # [128, 16, 2048]

    from contextlib import ExitStack
    with ExitStack() as _st:
        tc = _st.enter_context(TileContext(nc))
        constp = _st.enter_context(tc.tile_pool(name="const", bufs=1))
        abp = _st.enter_context(tc.tile_pool(name="ab", bufs=2))
        xsp = _st.enter_context(tc.tile_pool(name="xs", bufs=2))
        wqkp = _st.enter_context(tc.tile_pool(name="wqk", bufs=2))
        wvp = _st.enter_context(tc.tile_pool(name="wv", bufs=1))
        qkp = _st.enter_context(tc.tile_pool(name="qk", bufs=2))
        vsp = _st.enter_context(tc.tile_pool(name="vsb", bufs=2))
        wkp = _st.enter_context(tc.tile_pool(name="work", bufs=2))
        expp = _st.enter_context(tc.tile_pool(name="expp", bufs=2))
        otp = _st.enter_context(tc.tile_pool(name="outT", bufs=1))
        wosp = _st.enter_context(tc.tile_pool(name="wos", bufs=2))
        psproj = _st.enter_context(tc.tile_pool(name="ps_proj", bufs=2, space="PSUM"))
        pscp = _st.enter_context(tc.tile_pool(name="ps_sc", bufs=2, space="PSUM"))
        psse = _st.enter_context(tc.tile_pool(name="ps_se", bufs=1, space="PSUM"))
        pspv = _st.enter_context(tc.tile_pool(name="ps_pv", bufs=1, space="PSUM"))

        ones_b = constp.tile([P, P], BF16, tag="ones_b")
        nc.vector.memset(ones_b[:], 1.0)
        bias_q = constp.tile([P, 1], F32, tag="bias_q")
        nc.vector.memset(bias_q[:], DH * EPS)
        bias_k = constp.tile([P, 1], F32, tag="bias_k")
        nc.vector.memset(bias_k[:], EPS)

        outT = otp.tile([P, HPC, L], BF16, tag="outT")  # [d, head, l]

        # per-hp state handed from gen_proj to gen_attn
        state = {}

        def gen_proj(hp):
            """Projections + RMSNorm + RoPE for head-pair hp (2 heads)."""
            dh0 = hp * 2 * DH
            wqs = wqkp.tile([P, KT, 2 * DH], BF16, tag="wq")
            nc.sync.dma_start(wqs[:], wqv[:, :, dh0:dh0 + 2 * DH])
            wks = wqkp.tile([P, KT, 2 * DH], BF16, tag="wk")
            nc.sync.dma_start(wks[:], wkv[:, :, dh0:dh0 + 2 * DH])
            wvs = wvp.tile([P, KT, 2 * DH], BF16, tag="wvs")
            nc.sync.dma_start(wvs[:], wvv[:, :, dh0:dh0 + 2 * DH])

            qT = qkp.tile([P, 2, L], BF16, tag="qT")  # [dh, h2, l]
            kT = qkp.tile([P, 2, L], BF16, tag="kT")
            v_sb = vsp.tile([P, MT, 2 * DH], BF16, tag="vsb")
            state[hp] = (qT, kT, v_sb)

            for lc in range(NLCP):
                ls = lc * LCP
                xs = xsp.tile([P, KT, LCP], BF16, tag="xs")
                nc.sync.dma_start(xs[:], xv[:, :, ls:ls + LCP])
                abt = abp.tile([P, 4, LCP], BF16, tag="abt")
                nc.sync.dma_start(abt[:], abv[:, :, ls:ls + LCP])

                for h2 in range(2):
                    for which in range(2):  # 0 = q, 1 = k
                        w_sl = wqs if which == 0 else wks
                        dst = (qT if which == 0 else kT)[:, h2, ls:ls + LCP]
                        ps = psproj.tile([P, LCP], F32, tag="proj")
                        for kt in range(KT):
                            nc.tensor.matmul(
                                ps[:],
                                lhsT=w_sl[:, kt, h2 * DH:(h2 + 1) * DH],
                                rhs=xs[:, kt, :],
                                start=(kt == 0), stop=(kt == KT - 1),
                            )
                        sq = wkp.tile([P, LCP], BF16, tag="sq")
                        nc.scalar.activation(
                            sq[:], ps[:], AF.Square)
                        ss = psproj.tile([P, LCP], F32, tag="proj")
                        nc.tensor.matmul(ss[:], lhsT=ones_b[:], rhs=sq[:])
                        # rstd = 1/sqrt(scale*ss + bias) = Exp(-0.5*Ln(...))
                        # q side folds the 1/sqrt(DH) score scale.
                        if which == 0:
                            sc_, bi_ = 1.0, bias_q
                        else:
                            sc_, bi_ = 1.0 / DH, bias_k
                        lnt = wkp.tile([P, LCP], F32, tag="lnt")
                        nc.scalar.activation(lnt[:], ss[:], AF.Ln,
                                             bias=bi_[:], scale=sc_)
                        rstd = wkp.tile([P, LCP], BF16, tag="rstd")
                        nc.scalar.activation(rstd[:], lnt[:], AF.Exp,
                                             bias=0.0, scale=-0.5)
                        # RoPE: dst = (ps*A + rot(ps)*B) * rstd
                        a_sl = abt[:, 2 * which, :]
                        b_sl = abt[:, 2 * which + 1, :]
                        t1 = wkp.tile([P, LCP], BF16, tag="t1")
                        nc.vector.tensor_mul(t1[:], ps[:], a_sl)
                        t2 = wkp.tile([P, LCP], BF16, tag="t2")
                        nc.vector.tensor_mul(
                            t2[0:64, :], ps[64:128, :], b_sl[0:64, :])
                        nc.vector.tensor_mul(
                            t2[64:128, :], ps[0:64, :], b_sl[64:128, :])
                        nc.vector.tensor_add(t1[:], t1[:], t2[:])
                        nc.vector.tensor_mul(dst, t1[:], rstd[:])
                        yield

                # v projection for this l-chunk (natural [l, dh] layout)
                for sub in range(LCP // P):
                    lt = lc * (LCP // P) + sub
                    psv = psproj.tile([P, 2 * DH], F32, tag="proj")
                    for kt in range(KT):
                        nc.tensor.matmul(
                            psv[:],
                            lhsT=xs[:, kt, sub * P:(sub + 1) * P],
                            rhs=wvs[:, kt, :],
                            start=(kt == 0), stop=(kt == KT - 1),
                        )
                    nc.vector.tensor_copy(v_sb[:, lt, :], psv[:])
                    yield

        def gen_attn(hp):
            """Attention for head-pair hp; writes outT[:, hp*2+h2, :]."""
            qT, kT, v_sb = state[hp]
            for h2 in range(2):
                h = hp * 2 + h2
                for la in range(NLCA):
                    qs = la * LCA
                    ps_se = psse.tile([P, LCA], F32, tag="se")
                    ps_pv = pspv.tile([P, LCA], F32, tag="pv")
                    # software pipeline: scores group g+1 emitted before
                    # sum/PV of group g so the PE never waits on exp
                    prev = None
                    for g in range(MT // 2):
                        psc = pscp.tile([P, 2, LCA], F32, tag="sc")
                        for i in range(2):
                            mt = 2 * g + i
                            nc.tensor.matmul(
                                psc[:, i, :],
                                lhsT=kT[:, h2, mt * P:(mt + 1) * P],
                                rhs=qT[:, h2, qs:qs + LCA],
                            )
                        ex = expp.tile([P, 2, LCA], BF16, tag="ex")
                        nc.scalar.activation(ex[:], psc[:], AF.Exp)
                        if prev is not None:
                            pex, pg = prev
                            for i in range(2):
                                pmt = 2 * pg + i
                                nc.tensor.matmul(
                                    ps_se[:], lhsT=ones_b[:], rhs=pex[:, i, :],
                                    start=(pmt == 0), stop=False)
                                nc.tensor.matmul(
                                    ps_pv[:],
                                    lhsT=v_sb[:, pmt, h2 * DH:(h2 + 1) * DH],
                                    rhs=pex[:, i, :],
                                    start=(pmt == 0), stop=False)
                        prev = (ex, g)
                        yield
                    pex, pg = prev
                    for i in range(2):
                        pmt = 2 * pg + i
                        nc.tensor.matmul(
                            ps_se[:], lhsT=ones_b[:], rhs=pex[:, i, :],
                            start=False, stop=(i == 1))
                        nc.tensor.matmul(
                            ps_pv[:],
                            lhsT=v_sb[:, pmt, h2 * DH:(h2 + 1) * DH],
                            rhs=pex[:, i, :],
                            start=False, stop=(i == 1))
                    rec = wkp.tile([P, LCA], F32, tag="rec")
                    nc.vector.reciprocal_approx_fast(rec[:], ps_se[:])
                    nc.vector.tensor_mul(
                        outT[:, h, qs:qs + LCA], ps_pv[:], rec[:])
                    yield

        def gen_oproj():
            """o_proj partial over this core's 8 heads: out[l, n]."""
            for nch in range(4):
                ns = nch * 512
                wos = wosp.tile([P, HPC, 512], BF16, tag="wos")
                nc.sync.dma_start(wos[:], wov[:, :, ns:ns + 512])
                for lt in range(LT):
                    pso = psproj.tile([P, 512], F32, tag="proj")
                    for hh in range(HPC):
                        nc.tensor.matmul(
                            pso[:],
                            lhsT=outT[:, hh, lt * P:(lt + 1) * P],
                            rhs=wos[:, hh, :],
                            start=(hh == 0), stop=(hh == HPC - 1),
                        )
                    o_sb = wkp.tile([P, 512], F32, tag="osb")
                    nc.vector.tensor_copy(o_sb[:], pso[:])
                    nc.sync.dma_start(outv[:, lt, ns:ns + 512], o_sb[:])
                    yield

        def drain(g):
            for _ in g:
                pass

        def interleave(g_many, g_few, ratio):
            """Step g_many `ratio` times per single step of g_few."""
            done_many = done_few = False
            while not (done_many and done_few):
                for _ in range(ratio):
                    if not done_many:
                        done_many = next(g_many, "end") == "end"
                if not done_few:
                    done_few = next(g_few, "end") == "end"

        # prologue: projections for hp0 alone
        drain(gen_proj(0))
        # steady state: attention(hp) interleaved with projections(hp+1)
        for hp in range(3):
            interleave(gen_attn(hp), gen_proj(hp + 1), 2)
        # epilogue: attention(hp3) interleaved with the start of o_proj
        # (o_proj depends on all heads, so it can only run after attn(3)
        # blocks complete; emitting it interleaved lets the tail overlap)
        drain(gen_attn(3))
        drain(gen_oproj())

    nc.compile()
    return nc


def _host_prep(x, cos_q, sin_q, cos_k, sin_k, Wq, Wk, Wv, Wo, q_gamma, k_gamma):
    """Build the 8 per-core input maps."""
    f = np.float32
    bf = ml_dtypes.bfloat16
    sgn = np.concatenate([-np.ones(64, f), np.ones(64, f)])

    def ab_pair(cos_b, sin_b, gamma):
        grot = np.concatenate([gamma[64:], gamma[:64]])
        A = np.ascontiguousarray((cos_b * gamma[None, :]).T.astype(f))
        Bm = np.ascontiguousarray((sin_b * (sgn * grot)[None, :]).T.astype(f))
        return A, Bm

    in_maps = []
    for c in range(NCORES):
        b, hg = divmod(c, 2)
        sl = slice(hg * HPC * DH, (hg + 1) * HPC * DH)
        A_q, B_q = ab_pair(cos_q[b], sin_q[b], q_gamma)
        A_k, B_k = ab_pair(cos_k[b], sin_k[b], k_gamma)
        ab_all = np.stack([A_q, B_q, A_k, B_k]).astype(bf)
        in_maps.append({
            "xT": np.ascontiguousarray(x[b].T).astype(bf),
            "wqT": np.ascontiguousarray(Wq[sl, :].T).astype(bf),
            "wkT": np.ascontiguousarray(Wk[sl, :].T).astype(bf),
            "wvT": np.ascontiguousarray(Wv[sl, :].T).astype(bf),
            "woT": np.ascontiguousarray(Wo[:, sl].T).astype(bf),
            "ab": ab_all,
        })
    return in_maps


last_results = None


def kernel(x, cos_q, sin_q, cos_k, sin_k, Wq, Wk, Wv, Wo, q_gamma, k_gamma):
    global last_results
    if "nc" not in _cache:
        _cache["nc"] = _build_program()
    nc = _cache["nc"]
    args = [np.asarray(a) for a in (x, cos_q, sin_q, cos_k, sin_k,
                                    Wq, Wk, Wv, Wo, q_gamma, k_gamma)]
    in_maps = _host_prep(*args)
    trace = bool(int(os.environ.get("BASS_KERNEL_TRACE", "0")))
    try:
        res = run_bass_kernel_spmd(
            nc, in_maps, core_ids=list(range(NCORES)), trace=trace)
    except Exception:
        if not trace:
            raise
        res = run_bass_kernel_spmd(
            nc, in_maps, core_ids=list(range(NCORES)), trace=False)
    last_results = res
    outs = [r["out"] for r in res.results]
    full = np.empty((B, L, D), np.float32)
    for b in range(B):
        full[b] = outs[2 * b] + outs[2 * b + 1]
    return full
